# revision 8
# baseline (speedup 1.0000x reference)
"""Bidirectional cross-attention kernel for Trainium2, 8 NeuronCores.

Sharding: tensor-parallel over the 16 heads (2 heads per core). All
per-head work (QKV projections, shared similarity, both softmaxes, both
attention outputs) is head-local; each core computes its head-slice of
both attention outputs in transposed layout [head_dim, tokens] and
normalizes them. A per-batch AllToAll then routes, to each core, every
core's head-slice for the 128 token rows that core owns in that batch
(core c owns rows b*1024 + c*128 .. +128 of every batch b). Each core
finishes with the full output projection + biases + LayerNorm + residual
for its own 4x128 = 512 rows.

kernel(**inputs) takes the FULL unsharded inputs (as produced by
setup_inputs) and returns the FULL [4, 1024, 1024] output.
"""
import sys

sys.path.insert(0, "/opt/trn_rl_repo")

from contextlib import ExitStack

import numpy as np
import orjson

import concourse.bass as bass
import concourse.tile as tile
from concourse import mybir

# ---------------------------------------------------------------------------
# Workaround for this container's walrus build: it rejects any instruction
# carrying more than one sync wait. Post-process the serialized BIR (the
# single choke point used by both compile_bir_kernel and the bass2jax/PJRT
# lowering): an instruction with N>1 waits keeps the last one and gets N-1
# single-wait NoOps inserted right before it on the same engine. Waits gate
# issue, so hoisting them onto preceding same-engine NoOps is equivalent.
_orig_to_json_bytes = bass.Bass.to_json_bytes


def _split_waits(d):
    ctr = 0
    for fn in d.get("functions", []):
        for blk in fn.get("blocks", []):
            insts = blk.get("instructions")
            if not insts:
                continue
            out, changed = [], False
            for inst in insts:
                si = inst.get("sync_info")
                waits = (si or {}).get("on_wait") or []
                if len(waits) > 1:
                    changed = True
                    for w in waits[:-1]:
                        ctr += 1
                        out.append(
                            {
                                "name": f"I-ws{ctr}",
                                "opcode": "NoOp",
                                "engine": inst["engine"],
                                "ins": [],
                                "outs": [],
                                "debug": inst.get("debug"),
                                "sync_info": {"on_update": [], "on_wait": [w]},
                            }
                        )
                    si["on_wait"] = [waits[-1]]
                out.append(inst)
            if changed:
                blk["instructions"] = out
    return d


def _to_json_bytes_legal(self):
    return orjson.dumps(_split_waits(orjson.loads(_orig_to_json_bytes(self))))


if bass.Bass.to_json_bytes is not _to_json_bytes_legal:
    bass.Bass.to_json_bytes = _to_json_bytes_legal

# ---------------------------------------------------------------------------
B, N, D = 4, 1024, 1024
H, DH = 16, 64
R = B * N                 # 4096 token rows
NC = 8                    # cores
HPC = H // NC             # 2 heads per core
HD = HPC * DH             # 128-wide per-core head-dim slice
SCALE = DH ** -0.5

f32 = mybir.dt.float32
f32r = mybir.dt.float32r
f16 = mybir.dt.float16
AF = mybir.ActivationFunctionType
ALU = mybir.AluOpType

KT = D // 128             # 8 contraction tiles of 128 over the model dim
RT = N // 128             # 8 row tiles per batch
CK = N // 512             # 2 512-wide chunks per batch


def build_nc():
    nc = bass.Bass(num_devices=NC)

    xT_d = nc.dram_tensor("xT", [D, R], f32, kind="ExternalInput")
    cT_d = nc.dram_tensor("cT", [D, R], f32, kind="ExternalInput")
    xs_d = nc.dram_tensor("xs", [B * 128, D], f32, kind="ExternalInput")
    cs_d = nc.dram_tensor("cs", [B * 128, D], f32, kind="ExternalInput")
    wqk_d = nc.dram_tensor("wqk", [D, HD], f32, kind="ExternalInput")
    wcqk_d = nc.dram_tensor("wcqk", [D, HD], f32, kind="ExternalInput")
    wv_d = nc.dram_tensor("wv", [D, HD], f32, kind="ExternalInput")
    wcv_d = nc.dram_tensor("wcv", [D, HD], f32, kind="ExternalInput")
    wout_d = nc.dram_tensor("wout", [D, D], f32, kind="ExternalInput")
    wcout_d = nc.dram_tensor("wcout", [D, D], f32, kind="ExternalInput")
    bout_d = nc.dram_tensor("bout", [1, D], f32, kind="ExternalInput")
    bcout_d = nc.dram_tensor("bcout", [1, D], f32, kind="ExternalInput")
    gamma_d = nc.dram_tensor("gamma", [1, D], f32, kind="ExternalInput")
    beta_d = nc.dram_tensor("beta", [1, D], f32, kind="ExternalInput")
    ones_d = nc.dram_tensor("ones", [1, 128], f32, kind="ExternalInput")
    out_d = nc.dram_tensor("out", [B * 128, D], f32, kind="ExternalOutput")

    with ExitStack() as ctx:
        tc = ctx.enter_context(tile.TileContext(nc))
        cpool = ctx.enter_context(tc.tile_pool(name="const", bufs=1))
        dram = ctx.enter_context(tc.tile_pool(name="dram", bufs=4, space="DRAM"))

        ones1 = cpool.tile([1, 128], f32r, tag="ones")
        nc.sync.dma_start(ones1[:], ones_d[:].bitcast(f32r))

        # collective buffers, one pair per batch.
        # cc_in[b][j] = slot for core j: this core's head-slice of both
        # outputs for token rows j*128..(j+1)*128 of batch b.
        cc_in = [dram.tile([NC, 2, HD, 128], f32, tag="ccin", name=f"ccin{b}")
                 for b in range(B)]
        cc_out = [dram.tile([NC, 2, HD, 128], f32, tag="ccout", name=f"ccout{b}")
                  for b in range(B)]

        # ---------------- phases A+B, per batch ----------------
        with ExitStack() as ab:
            pw = ab.enter_context(tc.tile_pool(name="pw", bufs=1))
            pps = ab.enter_context(tc.tile_pool(name="pps", bufs=2, space="PSUM"))
            psim = ab.enter_context(tc.tile_pool(name="psim", bufs=3, space="PSUM"))
            pacc = ab.enter_context(tc.tile_pool(name="pacc", bufs=2, space="PSUM"))
            pbc = ab.enter_context(tc.tile_pool(name="pbc", bufs=1, space="PSUM"))
            pxT = ab.enter_context(tc.tile_pool(name="pxT", bufs=8))
            pqk = ab.enter_context(tc.tile_pool(name="pqk", bufs=2))
            pv = ab.enter_context(tc.tile_pool(name="pv", bufs=16))
            pE = ab.enter_context(tc.tile_pool(name="pE", bufs=6))
            pET = ab.enter_context(tc.tile_pool(name="pET", bufs=16))
            pn = ab.enter_context(tc.tile_pool(name="pn", bufs=4))

            wqk_sb, wcqk_sb, wv_sb, wcv_sb = [], [], [], []
            for name, dsrc, lst in (
                ("wqk", wqk_d, wqk_sb),
                ("wcqk", wcqk_d, wcqk_sb),
                ("wv", wv_d, wv_sb),
                ("wcv", wcv_d, wcv_sb),
            ):
                for k in range(KT):
                    t = pw.tile([128, HD], f32r, tag=f"{name}{k}")
                    nc.sync.dma_start(t[:], dsrc[k * 128:(k + 1) * 128, :].bitcast(f32r))
                    lst.append(t)

            for b in range(B):
                rsl = slice(b * N, (b + 1) * N)
                # ---- phase A: transposed activations in, projections ----
                xTt, cTt = [], []
                for k in range(KT):
                    t = pxT.tile([128, N], f32r, tag="xT")
                    nc.sync.dma_start(t[:], xT_d[k * 128:(k + 1) * 128, rsl].bitcast(f32r))
                    xTt.append(t)
                    t = pxT.tile([128, N], f32r, tag="cT")
                    nc.sync.dma_start(t[:], cT_d[k * 128:(k + 1) * 128, rsl].bitcast(f32r))
                    cTt.append(t)

                qkT = pqk.tile([128, N], f32r, tag="qkT")
                cqkT = pqk.tile([128, N], f32r, tag="cqkT")
                for dst, w_sb, src in ((qkT, wqk_sb, xTt), (cqkT, wcqk_sb, cTt)):
                    for ck in range(CK):
                        ps = pps.tile([128, 512], f32, tag="ps")
                        for k in range(KT):
                            nc.tensor.matmul(
                                ps[:], w_sb[k][:], src[k][:, ck * 512:(ck + 1) * 512],
                                start=(k == 0), stop=(k == KT - 1),
                            )
                        nc.vector.tensor_copy(dst[:, ck * 512:(ck + 1) * 512], ps[:])

                # v/cv natural, f16, with interleaved ones columns:
                # per r-tile: [v_h0 (0:64) | 1 | v_h1 (65:129) | 1]
                v1, cv1 = [], []
                for w_sb, src, lst, tg in (
                    (wv_sb, xTt, v1, "v1"),
                    (wcv_sb, cTt, cv1, "cv1"),
                ):
                    for rt in range(RT):
                        ps = pps.tile([128, 128], f32, tag="ps")
                        for k in range(KT):
                            nc.tensor.matmul(
                                ps[:], src[k][:, rt * 128:(rt + 1) * 128], w_sb[k][:],
                                start=(k == 0), stop=(k == KT - 1),
                            )
                        t = pv.tile([128, 130], f16, tag=tg)
                        nc.vector.tensor_copy(t[:, 0:64], ps[:, 0:64])
                        nc.vector.memset(t[:, 64:65], 1.0)
                        nc.vector.tensor_copy(t[:, 65:129], ps[:, 64:128])
                        nc.vector.memset(t[:, 129:130], 1.0)
                        lst.append(t)

                # ---- phase B direction 1 (context_out), builds E^T ----
                ET = [[pET.tile([128, N], f16, tag="ET", name=f"ET{b}_{h}_{ct}")
                       for ct in range(KT)] for h in range(2)]
                for ck in range(CK):
                    accs = [pacc.tile([128, 512], f32, tag="acc", name=f"acc{b}_{ck}_{h}")
                            for h in range(2)]
                    for rt in range(RT):
                        sims = []
                        for h in range(2):
                            ps_sim = psim.tile([128, 512], f32, tag="sim")
                            nc.tensor.matmul(
                                ps_sim[:],
                                qkT[h * 64:(h + 1) * 64, rt * 128:(rt + 1) * 128],
                                cqkT[h * 64:(h + 1) * 64, ck * 512:(ck + 1) * 512],
                                start=True, stop=True,
                                tile_position=(h * 64, 0),
                            )
                            sims.append(ps_sim)
                        for h in range(2):
                            E = pE.tile([128, 512], f16, tag="E")
                            nc.scalar.activation(E[:], sims[h][:], AF.Exp, scale=SCALE)
                            nc.tensor.matmul(
                                accs[h][0:65, :],
                                v1[rt][:, h * 65:(h + 1) * 65],
                                E[:],
                                start=(rt == 0), stop=(rt == RT - 1),
                            )
                            for j in range(4):
                                ct = ck * 4 + j
                                nc.scalar.dma_start_transpose(
                                    ET[h][ct][:, rt * 128:(rt + 1) * 128],
                                    E[:, j * 128:(j + 1) * 128],
                                )
                    for h in range(2):
                        _normalize_multi(
                            nc, pbc, pn, dram, accs[h], ones1,
                            [cc_in[b][ck * 4 + j, 1, h * 64:(h + 1) * 64, :] for j in range(4)],
                        )

                # ---- phase B direction 2 (out), consumes E^T ----
                for ck in range(CK):
                    for h in range(2):
                        acc = pacc.tile([128, 512], f32, tag="acc")
                        for ct in range(KT):
                            nc.tensor.matmul(
                                acc[0:65, :],
                                cv1[ct][:, h * 65:(h + 1) * 65],
                                ET[h][ct][:, ck * 512:(ck + 1) * 512],
                                start=(ct == 0), stop=(ct == KT - 1),
                            )
                        _normalize_multi(
                            nc, pbc, pn, dram, acc, ones1,
                            [cc_in[b][ck * 4 + j, 0, h * 64:(h + 1) * 64, :] for j in range(4)],
                        )

                nc.gpsimd.collective_compute(
                    "AllToAll",
                    ALU.bypass,
                    replica_groups=[list(range(NC))],
                    ins=[cc_in[b].opt()],
                    outs=[cc_out[b].opt()],
                )

        # ---------------- phase C: own-shard projection + LN ----------------
        with ExitStack() as pc:
            pwo = pc.enter_context(tc.tile_pool(name="pwo", bufs=1))
            psumP = pc.enter_context(tc.tile_pool(name="psumP", bufs=1, space="PSUM"))
            pbcC = pc.enter_context(tc.tile_pool(name="pbcC", bufs=1, space="PSUM"))
            pag = pc.enter_context(tc.tile_pool(name="pag", bufs=32))
            pln = pc.enter_context(tc.tile_pool(name="pln", bufs=2))

            wout_sb, wcout_sb = [], []
            for name, dsrc, lst in (("wo", wout_d, wout_sb), ("wco", wcout_d, wcout_sb)):
                for k in range(KT):
                    t = pwo.tile([128, D], f32r, tag=f"{name}{k}")
                    nc.sync.dma_start(t[:], dsrc[k * 128:(k + 1) * 128, :].bitcast(f32r))
                    lst.append(t)

            bout_r = cpool.tile([1, D], f32r, tag="bout")
            nc.sync.dma_start(bout_r[:], bout_d[:].bitcast(f32r))
            bcout_r = cpool.tile([1, D], f32r, tag="bcout")
            nc.sync.dma_start(bcout_r[:], bcout_d[:].bitcast(f32r))
            gamma_r = cpool.tile([1, D], f32r, tag="gamma")
            nc.sync.dma_start(gamma_r[:], gamma_d[:].bitcast(f32r))
            beta_r = cpool.tile([1, D], f32r, tag="beta")
            nc.sync.dma_start(beta_r[:], beta_d[:].bitcast(f32r))
            epsc = cpool.tile([128, 1], f32, tag="eps")
            nc.vector.memset(epsc[:], 1e-5)

            gamma_bc = cpool.tile([128, D], f32, tag="gbc")
            beta_bc = cpool.tile([128, D], f32, tag="bbc")
            for row_r, dst in ((gamma_r, gamma_bc), (beta_r, beta_bc)):
                for half in range(2):
                    psb = pbcC.tile([128, 512], f32, tag="bc")
                    nc.tensor.matmul(
                        psb[:], ones1[0:1, :], row_r[:, half * 512:(half + 1) * 512],
                        start=True, stop=True,
                    )
                    nc.vector.tensor_copy(dst[:, half * 512:(half + 1) * 512], psb[:])

            for b in range(B):
                ago, agc = [], []
                for k in range(NC):
                    t = pag.tile([128, 128], f32r, tag="ago")
                    nc.sync.dma_start(t[:], cc_out[b][k, 0, :, :].bitcast(f32r))
                    ago.append(t)
                    t = pag.tile([128, 128], f32r, tag="agc")
                    nc.sync.dma_start(t[:], cc_out[b][k, 1, :, :].bitcast(f32r))
                    agc.append(t)

                ps = psumP.tile([128, D], f32, tag="proj")
                for half in range(2):
                    hsl = slice(half * 512, (half + 1) * 512)
                    for k in range(NC):
                        nc.tensor.matmul(ps[:, hsl], ago[k][:], wout_sb[k][:, hsl],
                                         start=(k == 0), stop=False)
                    for k in range(NC):
                        nc.tensor.matmul(ps[:, hsl], agc[k][:], wcout_sb[k][:, hsl],
                                         start=False, stop=False)
                    nc.tensor.matmul(ps[:, hsl], ones1[0:1, :], bout_r[:, hsl],
                                     start=False, stop=False)
                    nc.tensor.matmul(ps[:, hsl], ones1[0:1, :], bcout_r[:, hsl],
                                     start=False, stop=True)

                # LayerNorm + gamma/beta + residual
                t_sb = pln.tile([128, D], f32, tag="t_sb")
                rsum = pln.tile([128, 1], f32, tag="rsum")
                nc.vector.tensor_scalar(t_sb[:], ps[:], 1.0, 0.0, ALU.mult, ALU.add,
                                        accum_out=rsum[:])
                tsq = pln.tile([128, D], f32, tag="tsq")
                ssq = pln.tile([128, 1], f32, tag="ssq")
                nc.vector.scalar_tensor_tensor(tsq[:], ps[:], 1.0, t_sb[:],
                                               ALU.mult, ALU.mult, accum_out=ssq[:])
                mean = pln.tile([128, 1], f32, tag="mean")
                nc.vector.tensor_scalar(mean[:], rsum[:], 1.0 / D, None, ALU.mult)
                msq = pln.tile([128, 1], f32, tag="msq")
                nc.vector.tensor_tensor(msq[:], mean[:], mean[:], ALU.mult)
                var = pln.tile([128, 1], f32, tag="var")
                nc.vector.tensor_scalar(var[:], ssq[:], 1.0 / D, msq[:], ALU.mult, ALU.subtract)
                std = pln.tile([128, 1], f32, tag="std")
                nc.scalar.activation(std[:], var[:], AF.Sqrt, bias=epsc[:])
                rstd = pln.tile([128, 1], f32, tag="rstd")
                nc.vector.reciprocal(rstd[:], std[:])

                nrm = pln.tile([128, D], f32, tag="nrm")
                nc.vector.tensor_scalar(nrm[:], t_sb[:], mean[:], rstd[:],
                                        ALU.subtract, ALU.mult)
                gm = pln.tile([128, D], f32, tag="gm")
                nc.vector.tensor_tensor(gm[:], nrm[:], gamma_bc[:], ALU.mult)

                xs_t = pln.tile([128, D], f32, tag="xs")
                nc.sync.dma_start(xs_t[:], xs_d[b * 128:(b + 1) * 128, :])
                cs_t = pln.tile([128, D], f32, tag="cs")
                nc.sync.dma_start(cs_t[:], cs_d[b * 128:(b + 1) * 128, :])
                rsb = pln.tile([128, D], f32, tag="rsb")
                nc.vector.scalar_tensor_tensor(rsb[:], xs_t[:], 1.0, cs_t[:],
                                               ALU.mult, ALU.add)
                rb2 = pln.tile([128, D], f32, tag="rb2")
                nc.vector.tensor_tensor(rb2[:], rsb[:], beta_bc[:], ALU.add)
                fin = pln.tile([128, D], f32, tag="fin")
                nc.vector.tensor_tensor(fin[:], gm[:], rb2[:], ALU.add)
                nc.sync.dma_start(out_d[b * 128:(b + 1) * 128, :], fin[:])

    return nc


def _normalize_multi(nc, pbc, pn, dram, acc, ones1, cc_slices):
    """Like _normalize but DMAs the normalized [64, 512] chunk to four
    [64, 128] DRAM slices (the per-destination-core A2A slots)."""
    srow = pn.tile([1, 512], f32, tag="srow")
    nc.scalar.activation(srow[:], acc[64:65, :], AF.Identity)
    dscr = dram.tile([512], f32, tag="dscr")
    nc.sync.dma_start(dscr[:].rearrange("(a b) -> a b", a=1), srow[0:1, :])
    scol = pn.tile([128, 4], f32, tag="scol")
    nc.sync.dma_start(scol[:, 0:4], dscr[:].rearrange("(a b) -> a b", b=4))
    rcol = pn.tile([128, 4], f32, tag="rcol")
    nc.vector.reciprocal(rcol[:], scol[:])
    dscr2 = dram.tile([512], f32, tag="dscr2")
    nc.sync.dma_start(dscr2[:].rearrange("(a b) -> a b", b=4), rcol[:, 0:4])
    rrow = pn.tile([1, 512], f32r, tag="rrow")
    nc.sync.dma_start(rrow[0:1, :], dscr2[:].rearrange("(a b) -> a b", a=1).bitcast(f32r))
    psb = pbc.tile([128, 512], f32, tag="bc")
    nc.tensor.matmul(psb[0:64, :], ones1[0:1, 0:64], rrow[:], start=True, stop=True)
    bcs = pn.tile([64, 512], f32, tag="bcs")
    nc.vector.tensor_copy(bcs[:], psb[0:64, :])
    outn = pn.tile([64, 512], f32, tag="outn")
    nc.vector.tensor_tensor(outn[:], acc[0:64, :], bcs[:], ALU.mult)
    for j, sl in enumerate(cc_slices):
        nc.sync.dma_start(sl, outn[:, j * 128:(j + 1) * 128])


_NC_CACHE = None


def _get_nc():
    global _NC_CACHE
    if _NC_CACHE is None:
        _NC_CACHE = build_nc()
    return _NC_CACHE


def kernel(x, context, W_qk, W_cqk, W_v, W_cv, W_out, b_out, W_cout, b_cout,
           gamma, beta):
    x = np.asarray(x, dtype=np.float32)
    context = np.asarray(context, dtype=np.float32)
    xf = x.reshape(R, D)
    cf = context.reshape(R, D)
    xT = np.ascontiguousarray(xf.T)
    cT = np.ascontiguousarray(cf.T)

    def shard_rows(a, c):
        # rows b*1024 + c*128 .. +128 for each batch b
        return np.ascontiguousarray(
            a.reshape(B, N, D)[:, c * 128:(c + 1) * 128, :].reshape(B * 128, D)
        )

    ones = np.ones((1, 128), np.float32)
    W_qk = np.asarray(W_qk, np.float32)
    W_cqk = np.asarray(W_cqk, np.float32)
    W_v = np.asarray(W_v, np.float32)
    W_cv = np.asarray(W_cv, np.float32)
    in_maps = []
    for c in range(NC):
        hs = slice(c * HD, (c + 1) * HD)
        in_maps.append({
            "xT": xT,
            "cT": cT,
            "xs": shard_rows(xf, c),
            "cs": shard_rows(cf, c),
            "wqk": np.ascontiguousarray(W_qk[:, hs]),
            "wcqk": np.ascontiguousarray(W_cqk[:, hs]),
            "wv": np.ascontiguousarray(W_v[:, hs]),
            "wcv": np.ascontiguousarray(W_cv[:, hs]),
            "wout": np.asarray(W_out, np.float32),
            "wcout": np.asarray(W_cout, np.float32),
            "bout": np.asarray(b_out, np.float32).reshape(1, D),
            "bcout": np.asarray(b_cout, np.float32).reshape(1, D),
            "gamma": np.asarray(gamma, np.float32).reshape(1, D),
            "beta": np.asarray(beta, np.float32).reshape(1, D),
            "ones": ones,
        })

    from concourse.bass_utils import run_bass_kernel_spmd

    res = run_bass_kernel_spmd(_get_nc(), in_maps, list(range(NC)))

    out = np.empty((B, N, D), np.float32)
    for c in range(NC):
        blk = res.results[c]["out"].reshape(B, 128, D)
        out[:, c * 128:(c + 1) * 128, :] = blk
    return out


if __name__ == "__main__":
    rng = np.random.default_rng(0)
    ins = {
        "x": rng.standard_normal((B, N, D)).astype(np.float32),
        "context": rng.standard_normal((B, N, D)).astype(np.float32),
        "W_qk": (rng.standard_normal((D, D)) * 0.02).astype(np.float32),
        "W_cqk": (rng.standard_normal((D, D)) * 0.02).astype(np.float32),
        "W_v": (rng.standard_normal((D, D)) * 0.02).astype(np.float32),
        "W_cv": (rng.standard_normal((D, D)) * 0.02).astype(np.float32),
        "W_out": (rng.standard_normal((D, D)) * 0.02).astype(np.float32),
        "b_out": (rng.standard_normal((D,)) * 0.02).astype(np.float32),
        "W_cout": (rng.standard_normal((D, D)) * 0.02).astype(np.float32),
        "b_cout": (rng.standard_normal((D,)) * 0.02).astype(np.float32),
        "gamma": np.ones((D,), np.float32),
        "beta": np.zeros((D,), np.float32),
    }
    out = kernel(**ins)
    print("kernel ran, out shape", out.shape, "mean", float(out.mean()))


# revision 9
# speedup vs baseline: 107.1148x; 107.1148x over previous
"""Bidirectional cross-attention kernel for Trainium2, 8 NeuronCores.

Sharding: tensor-parallel over the 16 heads (2 heads per core). All
per-head work (QKV projections, shared similarity, both softmaxes, both
attention outputs) is head-local; each core computes its head-slice of
both attention outputs in transposed layout [head_dim, tokens] and
normalizes them. A per-batch AllToAll then routes, to each core, every
core's head-slice for the 128 token rows that core owns in that batch
(core c owns rows b*1024 + c*128 .. +128 of every batch b). Each core
finishes with the full output projection + biases + LayerNorm + residual
for its own 4x128 = 512 rows.

kernel(**inputs) takes the FULL unsharded inputs (as produced by
setup_inputs) and returns the FULL [4, 1024, 1024] output.
"""
import sys

sys.path.insert(0, "/opt/trn_rl_repo")

from contextlib import ExitStack

import numpy as np
import orjson

import concourse.bass as bass
import concourse.tile as tile
from concourse import mybir

# ---------------------------------------------------------------------------
# Workaround for this container's walrus build: it rejects any instruction
# carrying more than one sync wait. Post-process the serialized BIR (the
# single choke point used by both compile_bir_kernel and the bass2jax/PJRT
# lowering): an instruction with N>1 waits keeps the last one and gets N-1
# single-wait NoOps inserted right before it on the same engine. Waits gate
# issue, so hoisting them onto preceding same-engine NoOps is equivalent.
_orig_to_json_bytes = bass.Bass.to_json_bytes


def _split_waits(d):
    ctr = 0
    for fn in d.get("functions", []):
        for blk in fn.get("blocks", []):
            insts = blk.get("instructions")
            if not insts:
                continue
            out, changed = [], False
            for inst in insts:
                si = inst.get("sync_info")
                waits = (si or {}).get("on_wait") or []
                if len(waits) > 1:
                    changed = True
                    for w in waits[:-1]:
                        ctr += 1
                        out.append(
                            {
                                "name": f"I-ws{ctr}",
                                "opcode": "NoOp",
                                "engine": inst["engine"],
                                "ins": [],
                                "outs": [],
                                "debug": inst.get("debug"),
                                "sync_info": {"on_update": [], "on_wait": [w]},
                            }
                        )
                    si["on_wait"] = [waits[-1]]
                out.append(inst)
            if changed:
                blk["instructions"] = out
    return d


def _to_json_bytes_legal(self):
    return orjson.dumps(_split_waits(orjson.loads(_orig_to_json_bytes(self))))


if bass.Bass.to_json_bytes is not _to_json_bytes_legal:
    bass.Bass.to_json_bytes = _to_json_bytes_legal

# ---------------------------------------------------------------------------
B, N, D = 4, 1024, 1024
H, DH = 16, 64
R = B * N                 # 4096 token rows
NC = 8                    # cores
HPC = H // NC             # 2 heads per core
HD = HPC * DH             # 128-wide per-core head-dim slice
SCALE = DH ** -0.5

f32 = mybir.dt.float32
f32r = mybir.dt.float32r
f16 = mybir.dt.float16
AF = mybir.ActivationFunctionType
ALU = mybir.AluOpType

KT = D // 128             # 8 contraction tiles of 128 over the model dim
RT = N // 128             # 8 row tiles per batch
CK = N // 512             # 2 512-wide chunks per batch


def build_nc():
    nc = bass.Bass(num_devices=NC)

    xT_d = nc.dram_tensor("xT", [D, R], f32, kind="ExternalInput")
    cT_d = nc.dram_tensor("cT", [D, R], f32, kind="ExternalInput")
    xs_d = nc.dram_tensor("xs", [B * 128, D], f32, kind="ExternalInput")
    cs_d = nc.dram_tensor("cs", [B * 128, D], f32, kind="ExternalInput")
    wqk_d = nc.dram_tensor("wqk", [D, HD], f32, kind="ExternalInput")
    wcqk_d = nc.dram_tensor("wcqk", [D, HD], f32, kind="ExternalInput")
    wv_d = nc.dram_tensor("wv", [D, HD], f32, kind="ExternalInput")
    wcv_d = nc.dram_tensor("wcv", [D, HD], f32, kind="ExternalInput")
    wout_d = nc.dram_tensor("wout", [D, D], f32, kind="ExternalInput")
    wcout_d = nc.dram_tensor("wcout", [D, D], f32, kind="ExternalInput")
    bout_d = nc.dram_tensor("bout", [1, D], f32, kind="ExternalInput")
    bcout_d = nc.dram_tensor("bcout", [1, D], f32, kind="ExternalInput")
    gamma_d = nc.dram_tensor("gamma", [1, D], f32, kind="ExternalInput")
    beta_d = nc.dram_tensor("beta", [1, D], f32, kind="ExternalInput")
    ones_d = nc.dram_tensor("ones", [1, 128], f32, kind="ExternalInput")
    out_d = nc.dram_tensor("out", [B * 128, D], f32, kind="ExternalOutput")

    with ExitStack() as ctx:
        tc = ctx.enter_context(tile.TileContext(nc))
        cpool = ctx.enter_context(tc.tile_pool(name="const", bufs=1))
        dram = ctx.enter_context(tc.tile_pool(name="dram", bufs=4, space="DRAM"))

        ones1 = cpool.tile([1, 128], f32r, tag="ones")
        nc.sync.dma_start(ones1[:], ones_d[:].bitcast(f32r))

        # collective buffers, one pair per batch.
        # cc_in[b][j] = slot for core j: this core's head-slice of both
        # outputs for token rows j*128..(j+1)*128 of batch b.
        cc_in = [dram.tile([NC, 2, HD, 128], f32, tag="ccin", name=f"ccin{b}")
                 for b in range(B)]
        cc_out = [dram.tile([NC, 2, HD, 128], f32, tag="ccout", name=f"ccout{b}")
                  for b in range(B)]

        # ---------------- phases A+B, per batch ----------------
        with ExitStack() as ab:
            pw = ab.enter_context(tc.tile_pool(name="pw", bufs=1))
            pps = ab.enter_context(tc.tile_pool(name="pps", bufs=2, space="PSUM"))
            psim = ab.enter_context(tc.tile_pool(name="psim", bufs=3, space="PSUM"))
            pacc = ab.enter_context(tc.tile_pool(name="pacc", bufs=2, space="PSUM"))
            pbc = ab.enter_context(tc.tile_pool(name="pbc", bufs=1, space="PSUM"))
            pxT = ab.enter_context(tc.tile_pool(name="pxT", bufs=8))
            pqk = ab.enter_context(tc.tile_pool(name="pqk", bufs=2))
            pv = ab.enter_context(tc.tile_pool(name="pv", bufs=16))
            pE = ab.enter_context(tc.tile_pool(name="pE", bufs=6))
            pET = ab.enter_context(tc.tile_pool(name="pET", bufs=16))
            pn = ab.enter_context(tc.tile_pool(name="pn", bufs=4))

            wqk_sb, wcqk_sb, wv_sb, wcv_sb = [], [], [], []
            for name, dsrc, lst in (
                ("wqk", wqk_d, wqk_sb),
                ("wcqk", wcqk_d, wcqk_sb),
                ("wv", wv_d, wv_sb),
                ("wcv", wcv_d, wcv_sb),
            ):
                for k in range(KT):
                    t = pw.tile([128, HD], f32r, tag=f"{name}{k}")
                    nc.sync.dma_start(t[:], dsrc[k * 128:(k + 1) * 128, :].bitcast(f32r))
                    lst.append(t)

            for b in range(B):
                rsl = slice(b * N, (b + 1) * N)
                # ---- phase A: transposed activations in, projections ----
                xTt, cTt = [], []
                for k in range(KT):
                    t = pxT.tile([128, N], f32r, tag="xT")
                    nc.sync.dma_start(t[:], xT_d[k * 128:(k + 1) * 128, rsl].bitcast(f32r))
                    xTt.append(t)
                    t = pxT.tile([128, N], f32r, tag="cT")
                    nc.sync.dma_start(t[:], cT_d[k * 128:(k + 1) * 128, rsl].bitcast(f32r))
                    cTt.append(t)

                qkT = pqk.tile([128, N], f32r, tag="qkT")
                cqkT = pqk.tile([128, N], f32r, tag="cqkT")
                for dst, w_sb, src in ((qkT, wqk_sb, xTt), (cqkT, wcqk_sb, cTt)):
                    for ck in range(CK):
                        ps = pps.tile([128, 512], f32, tag="ps")
                        for k in range(KT):
                            nc.tensor.matmul(
                                ps[:], w_sb[k][:], src[k][:, ck * 512:(ck + 1) * 512],
                                start=(k == 0), stop=(k == KT - 1),
                            )
                        nc.vector.tensor_copy(dst[:, ck * 512:(ck + 1) * 512], ps[:])

                # v/cv natural, f16, with interleaved ones columns:
                # per r-tile: [v_h0 (0:64) | 1 | v_h1 (65:129) | 1]
                v1, cv1 = [], []
                for w_sb, src, lst, tg in (
                    (wv_sb, xTt, v1, "v1"),
                    (wcv_sb, cTt, cv1, "cv1"),
                ):
                    for rt in range(RT):
                        ps = pps.tile([128, 128], f32, tag="ps")
                        for k in range(KT):
                            nc.tensor.matmul(
                                ps[:], src[k][:, rt * 128:(rt + 1) * 128], w_sb[k][:],
                                start=(k == 0), stop=(k == KT - 1),
                            )
                        t = pv.tile([128, 130], f16, tag=tg)
                        nc.vector.tensor_copy(t[:, 0:64], ps[:, 0:64])
                        nc.vector.memset(t[:, 64:65], 1.0)
                        nc.vector.tensor_copy(t[:, 65:129], ps[:, 64:128])
                        nc.vector.memset(t[:, 129:130], 1.0)
                        lst.append(t)

                # ---- phase B direction 1 (context_out), builds E^T ----
                ET = [[pET.tile([128, N], f16, tag="ET", name=f"ET{b}_{h}_{ct}")
                       for ct in range(KT)] for h in range(2)]
                for ck in range(CK):
                    accs = [pacc.tile([128, 512], f32, tag="acc", name=f"acc{b}_{ck}_{h}")
                            for h in range(2)]
                    for rt in range(RT):
                        sims = []
                        for h in range(2):
                            ps_sim = psim.tile([128, 512], f32, tag="sim")
                            nc.tensor.matmul(
                                ps_sim[:],
                                qkT[h * 64:(h + 1) * 64, rt * 128:(rt + 1) * 128],
                                cqkT[h * 64:(h + 1) * 64, ck * 512:(ck + 1) * 512],
                                start=True, stop=True,
                                tile_position=(h * 64, 0),
                            )
                            sims.append(ps_sim)
                        for h in range(2):
                            E = pE.tile([128, 512], f16, tag="E")
                            nc.scalar.activation(E[:], sims[h][:], AF.Exp, scale=SCALE)
                            nc.tensor.matmul(
                                accs[h][0:65, :],
                                v1[rt][:, h * 65:(h + 1) * 65],
                                E[:],
                                start=(rt == 0), stop=(rt == RT - 1),
                            )
                            for j in range(4):
                                ct = ck * 4 + j
                                nc.scalar.dma_start_transpose(
                                    ET[h][ct][:, rt * 128:(rt + 1) * 128],
                                    E[:, j * 128:(j + 1) * 128],
                                )
                    for h in range(2):
                        _normalize_multi(
                            nc, pbc, pn, dram, accs[h], ones1,
                            [cc_in[b][ck * 4 + j, 1, h * 64:(h + 1) * 64, :] for j in range(4)],
                        )

                # ---- phase B direction 2 (out), consumes E^T ----
                for ck in range(CK):
                    for h in range(2):
                        acc = pacc.tile([128, 512], f32, tag="acc")
                        for ct in range(KT):
                            nc.tensor.matmul(
                                acc[0:65, :],
                                cv1[ct][:, h * 65:(h + 1) * 65],
                                ET[h][ct][:, ck * 512:(ck + 1) * 512],
                                start=(ct == 0), stop=(ct == KT - 1),
                            )
                        _normalize_multi(
                            nc, pbc, pn, dram, acc, ones1,
                            [cc_in[b][ck * 4 + j, 0, h * 64:(h + 1) * 64, :] for j in range(4)],
                        )

                nc.gpsimd.collective_compute(
                    "AllToAll",
                    ALU.bypass,
                    replica_groups=[list(range(NC))],
                    ins=[cc_in[b].opt()],
                    outs=[cc_out[b].opt()],
                )

        # ---------------- phase C: own-shard projection + LN ----------------
        with ExitStack() as pc:
            pwo = pc.enter_context(tc.tile_pool(name="pwo", bufs=1))
            psumP = pc.enter_context(tc.tile_pool(name="psumP", bufs=1, space="PSUM"))
            pbcC = pc.enter_context(tc.tile_pool(name="pbcC", bufs=1, space="PSUM"))
            pag = pc.enter_context(tc.tile_pool(name="pag", bufs=32))
            pln = pc.enter_context(tc.tile_pool(name="pln", bufs=2))

            wout_sb, wcout_sb = [], []
            for name, dsrc, lst in (("wo", wout_d, wout_sb), ("wco", wcout_d, wcout_sb)):
                for k in range(KT):
                    t = pwo.tile([128, D], f32r, tag=f"{name}{k}")
                    nc.sync.dma_start(t[:], dsrc[k * 128:(k + 1) * 128, :].bitcast(f32r))
                    lst.append(t)

            bout_r = cpool.tile([1, D], f32r, tag="bout")
            nc.sync.dma_start(bout_r[:], bout_d[:].bitcast(f32r))
            bcout_r = cpool.tile([1, D], f32r, tag="bcout")
            nc.sync.dma_start(bcout_r[:], bcout_d[:].bitcast(f32r))
            gamma_r = cpool.tile([1, D], f32r, tag="gamma")
            nc.sync.dma_start(gamma_r[:], gamma_d[:].bitcast(f32r))
            beta_r = cpool.tile([1, D], f32r, tag="beta")
            nc.sync.dma_start(beta_r[:], beta_d[:].bitcast(f32r))
            epsc = cpool.tile([128, 1], f32, tag="eps")
            nc.vector.memset(epsc[:], 1e-5)

            gamma_bc = cpool.tile([128, D], f32, tag="gbc")
            beta_bc = cpool.tile([128, D], f32, tag="bbc")
            for row_r, dst in ((gamma_r, gamma_bc), (beta_r, beta_bc)):
                for half in range(2):
                    psb = pbcC.tile([128, 512], f32, tag="bc")
                    nc.tensor.matmul(
                        psb[:], ones1[0:1, :], row_r[:, half * 512:(half + 1) * 512],
                        start=True, stop=True,
                    )
                    nc.vector.tensor_copy(dst[:, half * 512:(half + 1) * 512], psb[:])

            for b in range(B):
                ago, agc = [], []
                for k in range(NC):
                    t = pag.tile([128, 128], f32r, tag="ago")
                    nc.sync.dma_start(t[:], cc_out[b][k, 0, :, :].bitcast(f32r))
                    ago.append(t)
                    t = pag.tile([128, 128], f32r, tag="agc")
                    nc.sync.dma_start(t[:], cc_out[b][k, 1, :, :].bitcast(f32r))
                    agc.append(t)

                ps = psumP.tile([128, D], f32, tag="proj")
                for half in range(2):
                    hsl = slice(half * 512, (half + 1) * 512)
                    for k in range(NC):
                        nc.tensor.matmul(ps[:, hsl], ago[k][:], wout_sb[k][:, hsl],
                                         start=(k == 0), stop=False)
                    for k in range(NC):
                        nc.tensor.matmul(ps[:, hsl], agc[k][:], wcout_sb[k][:, hsl],
                                         start=False, stop=False)
                    nc.tensor.matmul(ps[:, hsl], ones1[0:1, :], bout_r[:, hsl],
                                     start=False, stop=False)
                    nc.tensor.matmul(ps[:, hsl], ones1[0:1, :], bcout_r[:, hsl],
                                     start=False, stop=True)

                # LayerNorm + gamma/beta + residual
                t_sb = pln.tile([128, D], f32, tag="t_sb")
                rsum = pln.tile([128, 1], f32, tag="rsum")
                nc.vector.tensor_scalar(t_sb[:], ps[:], 1.0, 0.0, ALU.mult, ALU.add,
                                        accum_out=rsum[:])
                tsq = pln.tile([128, D], f32, tag="tsq")
                ssq = pln.tile([128, 1], f32, tag="ssq")
                nc.vector.scalar_tensor_tensor(tsq[:], ps[:], 1.0, t_sb[:],
                                               ALU.mult, ALU.mult, accum_out=ssq[:])
                mean = pln.tile([128, 1], f32, tag="mean")
                nc.vector.tensor_scalar(mean[:], rsum[:], 1.0 / D, None, ALU.mult)
                msq = pln.tile([128, 1], f32, tag="msq")
                nc.vector.tensor_tensor(msq[:], mean[:], mean[:], ALU.mult)
                var = pln.tile([128, 1], f32, tag="var")
                nc.vector.tensor_scalar(var[:], ssq[:], 1.0 / D, msq[:], ALU.mult, ALU.subtract)
                std = pln.tile([128, 1], f32, tag="std")
                nc.scalar.activation(std[:], var[:], AF.Sqrt, bias=epsc[:])
                rstd = pln.tile([128, 1], f32, tag="rstd")
                nc.vector.reciprocal(rstd[:], std[:])

                nrm = pln.tile([128, D], f32, tag="nrm")
                nc.vector.tensor_scalar(nrm[:], t_sb[:], mean[:], rstd[:],
                                        ALU.subtract, ALU.mult)
                gm = pln.tile([128, D], f32, tag="gm")
                nc.vector.tensor_tensor(gm[:], nrm[:], gamma_bc[:], ALU.mult)

                xs_t = pln.tile([128, D], f32, tag="xs")
                nc.sync.dma_start(xs_t[:], xs_d[b * 128:(b + 1) * 128, :])
                cs_t = pln.tile([128, D], f32, tag="cs")
                nc.sync.dma_start(cs_t[:], cs_d[b * 128:(b + 1) * 128, :])
                rsb = pln.tile([128, D], f32, tag="rsb")
                nc.vector.scalar_tensor_tensor(rsb[:], xs_t[:], 1.0, cs_t[:],
                                               ALU.mult, ALU.add)
                rb2 = pln.tile([128, D], f32, tag="rb2")
                nc.vector.tensor_tensor(rb2[:], rsb[:], beta_bc[:], ALU.add)
                fin = pln.tile([128, D], f32, tag="fin")
                nc.vector.tensor_tensor(fin[:], gm[:], rb2[:], ALU.add)
                nc.sync.dma_start(out_d[b * 128:(b + 1) * 128, :], fin[:])

    return nc


def _normalize_multi(nc, pbc, pn, dram, acc, ones1, cc_slices):
    """Like _normalize but DMAs the normalized [64, 512] chunk to four
    [64, 128] DRAM slices (the per-destination-core A2A slots)."""
    srow = pn.tile([1, 512], f32, tag="srow")
    nc.scalar.activation(srow[:], acc[64:65, :], AF.Identity)
    dscr = dram.tile([512], f32, tag="dscr")
    nc.sync.dma_start(dscr[:].rearrange("(a b) -> a b", a=1), srow[0:1, :])
    scol = pn.tile([128, 4], f32, tag="scol")
    nc.sync.dma_start(scol[:, 0:4], dscr[:].rearrange("(a b) -> a b", b=4))
    rcol = pn.tile([128, 4], f32, tag="rcol")
    nc.vector.reciprocal(rcol[:], scol[:])
    dscr2 = dram.tile([512], f32, tag="dscr2")
    nc.sync.dma_start(dscr2[:].rearrange("(a b) -> a b", b=4), rcol[:, 0:4])
    rrow = pn.tile([1, 512], f32r, tag="rrow")
    nc.sync.dma_start(rrow[0:1, :], dscr2[:].rearrange("(a b) -> a b", a=1).bitcast(f32r))
    psb = pbc.tile([128, 512], f32, tag="bc")
    nc.tensor.matmul(psb[0:64, :], ones1[0:1, 0:64], rrow[:], start=True, stop=True)
    bcs = pn.tile([64, 512], f32, tag="bcs")
    nc.vector.tensor_copy(bcs[:], psb[0:64, :])
    outn = pn.tile([64, 512], f32, tag="outn")
    nc.vector.tensor_tensor(outn[:], acc[0:64, :], bcs[:], ALU.mult)
    for j, sl in enumerate(cc_slices):
        nc.sync.dma_start(sl, outn[:, j * 128:(j + 1) * 128])


_NC_CACHE = None


def _get_nc():
    global _NC_CACHE
    if _NC_CACHE is None:
        _NC_CACHE = build_nc()
    return _NC_CACHE


def make_in_maps(x, context, W_qk, W_cqk, W_v, W_cv, W_out, b_out, W_cout,
                 b_cout, gamma, beta):
    x = np.asarray(x, dtype=np.float32)
    context = np.asarray(context, dtype=np.float32)
    xf = x.reshape(R, D)
    cf = context.reshape(R, D)
    xT = np.ascontiguousarray(xf.T)
    cT = np.ascontiguousarray(cf.T)

    def shard_rows(a, c):
        # rows b*1024 + c*128 .. +128 for each batch b
        return np.ascontiguousarray(
            a.reshape(B, N, D)[:, c * 128:(c + 1) * 128, :].reshape(B * 128, D)
        )

    ones = np.ones((1, 128), np.float32)
    W_qk = np.asarray(W_qk, np.float32)
    W_cqk = np.asarray(W_cqk, np.float32)
    W_v = np.asarray(W_v, np.float32)
    W_cv = np.asarray(W_cv, np.float32)
    in_maps = []
    for c in range(NC):
        hs = slice(c * HD, (c + 1) * HD)
        in_maps.append({
            "xT": xT,
            "cT": cT,
            "xs": shard_rows(xf, c),
            "cs": shard_rows(cf, c),
            "wqk": np.ascontiguousarray(W_qk[:, hs]),
            "wcqk": np.ascontiguousarray(W_cqk[:, hs]),
            "wv": np.ascontiguousarray(W_v[:, hs]),
            "wcv": np.ascontiguousarray(W_cv[:, hs]),
            "wout": np.asarray(W_out, np.float32),
            "wcout": np.asarray(W_cout, np.float32),
            "bout": np.asarray(b_out, np.float32).reshape(1, D),
            "bcout": np.asarray(b_cout, np.float32).reshape(1, D),
            "gamma": np.asarray(gamma, np.float32).reshape(1, D),
            "beta": np.asarray(beta, np.float32).reshape(1, D),
            "ones": ones,
        })
    return in_maps


def kernel(**inputs):
    in_maps = make_in_maps(**inputs)
    from concourse.bass_utils import run_bass_kernel_spmd

    res = run_bass_kernel_spmd(_get_nc(), in_maps, list(range(NC)))

    out = np.empty((B, N, D), np.float32)
    for c in range(NC):
        blk = res.results[c]["out"].reshape(B, 128, D)
        out[:, c * 128:(c + 1) * 128, :] = blk
    return out


if __name__ == "__main__":
    rng = np.random.default_rng(0)
    ins = {
        "x": rng.standard_normal((B, N, D)).astype(np.float32),
        "context": rng.standard_normal((B, N, D)).astype(np.float32),
        "W_qk": (rng.standard_normal((D, D)) * 0.02).astype(np.float32),
        "W_cqk": (rng.standard_normal((D, D)) * 0.02).astype(np.float32),
        "W_v": (rng.standard_normal((D, D)) * 0.02).astype(np.float32),
        "W_cv": (rng.standard_normal((D, D)) * 0.02).astype(np.float32),
        "W_out": (rng.standard_normal((D, D)) * 0.02).astype(np.float32),
        "b_out": (rng.standard_normal((D,)) * 0.02).astype(np.float32),
        "W_cout": (rng.standard_normal((D, D)) * 0.02).astype(np.float32),
        "b_cout": (rng.standard_normal((D,)) * 0.02).astype(np.float32),
        "gamma": np.ones((D,), np.float32),
        "beta": np.zeros((D,), np.float32),
    }
    out = kernel(**ins)
    print("kernel ran, out shape", out.shape, "mean", float(out.mean()))
